# revision 1
# baseline (speedup 1.0000x reference)
"""Trainium2 Bass kernel for nn_EndPointSpline.

Reference computation (per batch column b, feature d):
    xt = concat([x0, knots_b, x1])           # [T=128] knot values
    t  = spline_discr[:, b]                  # [T] sorted, t[0]=0, t[-1]=1
    vel[j] = (xt[j+1]-xt[j]) / (t[j+1]-t[j]+1e-10)
    left(q) = searchsorted(t[1:], q, 'left') clipped to [0, T-2]
    y(q) = xt[left] + vel[left] * (q - t[left])

Kernel strategy (data-parallel over B across 8 cores, 16 columns/core):
  Linear interpolation is expressed with P1 hat-function weights so the
  gather becomes ONE K=128 float32r matmul per query tile:
      r[j]    = 1/(t[j+1]-t[j]+1e-10)
      e1[i,q] = (q - t[i-1]) * r[i-1]     (row 0 uses sentinel -1/1)
      e2[i,q] = (t[i+1] - q) * r[i]       (row 127 uses sentinel 2/1)
      lam[i,q]= min(relu(e1), relu(e2))   -> lerp weights, 2 nonzeros per q
      y[q,d]  = sum_i lam[i,q] * xt[i,d]
  float32r gives full fp32-grade precision at 1 PE cycle/row for N=512.
  e1/e2/min run on DVE, the relu (+rounding to f32r) on ACT, PSUM is
  evacuated by DVE/ACT copies and written out in 2MB DMAs on the SP ring.

  Host-side marshalling: xt is pre-assembled to [B, T, D] (one contiguous
  input DMA per b), and queries are permuted within 1024-blocks so that each
  output partition writes a 16KB-contiguous DRAM run (the kernel writes the
  output in ORIGINAL query order; only the compute order is permuted).
"""

import numpy as np

Q, B, T, D = 2048, 128, 128, 512
NCORES = 8
BL = B // NCORES          # 16 batch columns per core
K = T - 1                 # 127 segments / contraction dim
NQT = Q // 128            # 16 query tiles of 128
GQT = 8                   # query tiles per output DMA group (2MB transfers)
NG = NQT // GQT           # output groups per b
PGROUP = GQT * 128        # queries per output group (1024)

_PROGRAM = None


def permute_queries(query_t):
    """qperm[g*PGROUP + k*128 + p] = query_t[g*PGROUP + p*GQT + k]."""
    a = np.asarray(query_t, dtype=np.float32).reshape(Q // PGROUP, 128, GQT)
    return np.ascontiguousarray(a.transpose(0, 2, 1).reshape(-1))


def assemble_xt(knots, x0, x1):
    """[B, T, D]: rows 0 / 1..T-2 / T-1 = x0 / knots / x1 per batch column."""
    return np.concatenate(
        [
            np.asarray(x0, dtype=np.float32).transpose(1, 0, 2),
            np.asarray(knots, dtype=np.float32),
            np.asarray(x1, dtype=np.float32).transpose(1, 0, 2),
        ],
        axis=1,
    )


def make_core_inputs(query_t, knots, x0, x1, spline_discr, core):
    """Per-core in_map for the Bass program (applies all host marshalling)."""
    s = slice(core * BL, (core + 1) * BL)
    xt_full = assemble_xt(knots[s], x0[:, s], x1[:, s])
    return {
        "query_t": permute_queries(query_t),
        "xt": np.ascontiguousarray(xt_full, dtype=np.float32),
        "spline_discr": np.ascontiguousarray(
            np.asarray(spline_discr, dtype=np.float32)[:, s]
        ),
    }


def _build_program(reps=1):
    import concourse.tile as tile
    from concourse import bacc, mybir

    f32 = mybir.dt.float32
    f32r = mybir.dt.float32r
    Alu = mybir.AluOpType
    Act = mybir.ActivationFunctionType

    nc = bacc.Bacc("TRN2", target_bir_lowering=False, debug=False)

    q_d = nc.dram_tensor("query_t", [Q], f32, kind="ExternalInput").ap()
    xt_d = nc.dram_tensor("xt", [BL, T, D], f32r, kind="ExternalInput").ap()
    t_d = nc.dram_tensor("spline_discr", [T, BL], f32, kind="ExternalInput").ap()
    out_d = nc.dram_tensor("out", [BL, Q, D], f32, kind="ExternalOutput").ap()

    with tile.TileContext(nc) as tc:
        with (
            tc.tile_pool(name="const", bufs=1) as cpool,
            tc.tile_pool(name="bwork", bufs=3) as bpool,
            tc.tile_pool(name="onehot", bufs=2) as opool,
            tc.tile_pool(name="outsb", bufs=3) as outpool,
            tc.tile_pool(name="psum", bufs=4, space="PSUM") as pspool,
        ):
            # --- per-core constants ---
            # qb[p, q] = permuted query_t[q] replicated across 128 partitions
            qb = cpool.tile([T, Q], f32)
            nc.scalar.dma_start(out=qb[:], in_=q_d.partition_broadcast(T))
            # t columns and r = 1/(dt+1e-10)
            tlo = cpool.tile([K, BL], f32)
            nc.scalar.dma_start(out=tlo[:], in_=t_d[0:K, :])
            thi = cpool.tile([K, BL], f32)
            nc.scalar.dma_start(out=thi[:], in_=t_d[1:T, :])
            r = cpool.tile([K, BL], f32)
            nc.vector.tensor_tensor(out=r[:], in0=thi[:], in1=tlo[:], op=Alu.subtract)
            nc.vector.tensor_scalar_add(out=r[:], in0=r[:], scalar1=1e-10)
            nc.vector.reciprocal(out=r[:], in_=r[:])
            # hat-function node constants:
            # tA[i]=t[i-1] (row0 -1), rA[i]=r[i-1] (row0 1),
            # tB[i]=t[i+1] (row127 2), nrB[i]=-r[i] (row127 -1)
            tA = cpool.tile([T, BL], f32)
            nc.vector.memset(tA[:], -1.0)
            nc.scalar.dma_start(out=tA[1:T, :], in_=t_d[0:K, :])
            rA = cpool.tile([T, BL], f32)
            nc.vector.memset(rA[:], 1.0)
            nc.sync.dma_start(out=rA[1:T, :], in_=r[:])
            tB = cpool.tile([T, BL], f32)
            nc.vector.memset(tB[:], 2.0)
            nc.scalar.dma_start(out=tB[0:K, :], in_=t_d[1:T, :])
            nrB = cpool.tile([T, BL], f32)
            nc.vector.memset(nrB[:], -1.0)
            nc.vector.tensor_scalar_mul(out=nrB[0:K, :], in0=r[:], scalar1=-1.0)

            for rep in range(reps):
                for b in range(BL):
                    # xt rows 0..127 in one contiguous DMA (SWDGE ring)
                    xf = bpool.tile([T, D], f32r)
                    nc.gpsimd.dma_start(out=xf[:], in_=xt_d[b, :, :])

                    # hat weights over all 2048 queries at once
                    e1 = opool.tile([T, Q], f32)
                    nc.vector.tensor_scalar(
                        out=e1[:], in0=qb[:], scalar1=tA[:, b : b + 1],
                        scalar2=rA[:, b : b + 1], op0=Alu.subtract, op1=Alu.mult,
                    )
                    e2 = opool.tile([T, Q], f32)
                    nc.vector.tensor_scalar(
                        out=e2[:], in0=qb[:], scalar1=tB[:, b : b + 1],
                        scalar2=nrB[:, b : b + 1], op0=Alu.subtract, op1=Alu.mult,
                    )
                    mn = opool.tile([T, Q], f32)
                    nc.vector.tensor_tensor(out=mn[:], in0=e1[:], in1=e2[:], op=Alu.min)
                    lam = opool.tile([T, Q], f32r)
                    nc.scalar.activation(out=lam[:], in_=mn[:], func=Act.Relu)

                    for g in range(NG):
                        osb = outpool.tile([128, GQT * D], f32)
                        for k2 in range(GQT // 2):
                            ps = pspool.tile([128, 2 * D], f32)
                            for half in range(2):
                                qt = g * GQT + k2 * 2 + half
                                sl = slice(qt * 128, (qt + 1) * 128)
                                nc.tensor.matmul(
                                    ps[:, half * D : (half + 1) * D],
                                    lhsT=lam[:, sl], rhs=xf[:],
                                    start=True, stop=True,
                                )
                            # evacuate 2 PSUM banks per op, split DVE/ACT
                            dst = osb[:, k2 * 2 * D : (k2 + 1) * 2 * D]
                            if (g * (GQT // 2) + k2) % 3 == 1:
                                nc.vector.tensor_copy(out=dst, in_=ps[:])
                            else:
                                nc.scalar.copy(out=dst, in_=ps[:])
                            first = rep == 0 and b == 0 and g == 0
                            last = (
                                rep == reps - 1 and b == BL - 1 and g == NG - 1
                            )
                            if first or last:
                                # ramp/tail: pair-granularity DMAs on the very
                                # first group (ring starts earlier) and the
                                # very last (drain overlaps the final copies)
                                fsl = slice(k2 * 2 * D, (k2 + 1) * 2 * D)
                                fview = out_d[
                                    b, g * PGROUP : (g + 1) * PGROUP, :
                                ].rearrange("(p c) d -> p (c d)", p=128)
                                nc.sync.dma_start(
                                    out=fview[:, fsl], in_=osb[:, fsl]
                                )
                        if (rep == 0 and b == 0 and g == 0) or (
                            rep == reps - 1 and b == BL - 1 and g == NG - 1
                        ):
                            continue
                        # one 2MB DMA per group on the SP ring; thanks to the
                        # query permutation each partition writes a 16KB run
                        dview = out_d[
                            b, g * PGROUP : (g + 1) * PGROUP, :
                        ].rearrange("(p c) d -> p (c d)", p=128)
                        nc.sync.dma_start(out=dview, in_=osb[:])
    nc.finalize()
    return nc


def _get_program(reps=1):
    global _PROGRAM
    if _PROGRAM is None:
        _PROGRAM = {}
    if reps not in _PROGRAM:
        _PROGRAM[reps] = _build_program(reps)
    return _PROGRAM[reps]


def kernel(query_t, knots, x0, x1, spline_discr, _trace=False, **_trace_kwargs):
    from concourse.bass_utils import run_bass_kernel_spmd

    query_t = np.asarray(query_t, dtype=np.float32)
    knots = np.asarray(knots, dtype=np.float32)
    x0 = np.asarray(x0, dtype=np.float32)
    x1 = np.asarray(x1, dtype=np.float32)
    spline_discr = np.asarray(spline_discr, dtype=np.float32)

    nc = _get_program()
    in_maps = [
        make_core_inputs(query_t, knots, x0, x1, spline_discr, c)
        for c in range(NCORES)
    ]
    res = run_bass_kernel_spmd(
        nc, in_maps, core_ids=list(range(NCORES)), trace=_trace, **_trace_kwargs
    )
    out = np.concatenate([r["out"] for r in res.results], axis=0)
    if _trace:
        return out, res
    return out



# revision 3
# speedup vs baseline: 2.2127x; 2.2127x over previous
"""Trainium2 Bass kernel for nn_EndPointSpline.

Reference computation (per batch column b, feature d):
    xt = concat([x0, knots_b, x1])           # [T=128] knot values
    t  = spline_discr[:, b]                  # [T] sorted, t[0]=0, t[-1]=1
    vel[j] = (xt[j+1]-xt[j]) / (t[j+1]-t[j]+1e-10)
    left(q) = searchsorted(t[1:], q, 'left') clipped to [0, T-2]
    y(q) = xt[left] + vel[left] * (q - t[left])

Kernel strategy (data-parallel over B across 8 cores, 16 columns/core):
  Piecewise-linear interpolation in *integrated* form: with segment
  lengths dt'[j] = t[j+1]-t[j]+1e-10 and slopes vel[j] = dxt[j]/dt'[j],
      y(q) = x0 + sum_j vel[j] * clamp(q - t[j], 0, dt'[j])
  The clamp weights w[j,q] = min(relu(q - t[j]), dt'[j]) are built in TWO
  fused DVE tensor_scalar ops (both 2x_2P fp32) over [128, 2048]:
      w  = max(q - tsh[i], 0)   (tsh[0] = -1 sentinel -> row 0 == 1)
      w  = min(w, dsh[i])       (dsh[0] = +1 sentinel -> row 0 == 1)
  so row 0 weights are exactly 1 and row 0 of A holds x0; rows 1..127 of
  A hold vel[0..126] (host-computed in f64, cast f32 -> every product
  vel*clamp is bounded by the actual knot increment: no cancellation).
  One K=128 float32r matmul per query tile gathers + lerps in one pass.

  PSUM (f32) is evacuated by DVE/ACT copies (ratio 5:11, balancing the
  two DVE weight ops against ACT) that downcast to bf16; the device
  writes a bf16 [BL, Q, D] output (halves HBM write traffic) and the
  host upcasts to f32. bf16 rounding adds ~2e-3 rel err vs the 2e-2
  budget.

  Host-side marshalling: queries are permuted so each output partition
  writes a 16KB-contiguous DRAM run, giving ONE 2MiB output DMA per b
  (the kernel writes the output in ORIGINAL query order; only the
  compute order is permuted).
"""

import numpy as np

Q, B, T, D = 2048, 128, 128, 512
NCORES = 8
BL = B // NCORES          # 16 batch columns per core
K = T - 1                 # 127 segments
NQT = Q // 128            # 16 query tiles of 128
PS_QT = 4                 # query tiles per PSUM tile (4 banks)
NBLK = NQT // PS_QT       # evac blocks per b

_PROGRAM = None


def permute_queries(query_t):
    """qperm[k*128 + p] = query_t[p*NQT + k] so that output partition p
    holds queries p*16..p*16+15 (a 16KB bf16 DRAM run)."""
    a = np.asarray(query_t, dtype=np.float32).reshape(128, NQT)
    return np.ascontiguousarray(a.T.reshape(-1))


def host_prep(query_t, knots, x0, x1, spline_discr):
    """Everything that is shared across cores (f64 math, cast f32)."""
    xt = np.concatenate(
        [
            np.asarray(x0, dtype=np.float32).transpose(1, 0, 2),
            np.asarray(knots, dtype=np.float32),
            np.asarray(x1, dtype=np.float32).transpose(1, 0, 2),
        ],
        axis=1,
    ).astype(np.float64)                                   # [B, T, D]
    t64 = np.asarray(spline_discr, dtype=np.float32).astype(np.float64)
    dtp64 = (t64[1:] - t64[:-1]) + 1e-10                   # [K, B]
    vel = (xt[:, 1:, :] - xt[:, :-1, :]) / dtp64.T[:, :, None]
    A = np.empty((B, T, D), np.float32)
    A[:, 0] = xt[:, 0]
    A[:, 1:] = vel                                         # [B, T, D]

    # shifted t / dt' with row-0 sentinels (-1 / +1 make row-0 weight == 1)
    tsh = np.empty((T, B), np.float32)
    tsh[0] = -1.0
    tsh[1:] = np.asarray(spline_discr, dtype=np.float32)[:K]
    dsh = np.empty((T, B), np.float32)
    dsh[0] = 1.0
    dsh[1:] = dtp64.astype(np.float32)
    return A, tsh, dsh


def make_core_inputs(qperm, A, tsh, dsh, core):
    s = slice(core * BL, (core + 1) * BL)
    return {
        "query_t": qperm,
        "amat": np.ascontiguousarray(A[s]),
        "tsh": np.ascontiguousarray(tsh[:, s]),
        "dsh": np.ascontiguousarray(dsh[:, s]),
    }


def _build_program(reps=1):
    import concourse.tile as tile
    from concourse import bacc, mybir

    f32 = mybir.dt.float32
    f32r = mybir.dt.float32r
    bf16 = mybir.dt.bfloat16
    Alu = mybir.AluOpType

    nc = bacc.Bacc("TRN2", target_bir_lowering=False, debug=False)

    q_d = nc.dram_tensor("query_t", [Q], f32, kind="ExternalInput").ap()
    a_d = nc.dram_tensor("amat", [BL, T, D], f32r, kind="ExternalInput").ap()
    tsh_d = nc.dram_tensor("tsh", [T, BL], f32, kind="ExternalInput").ap()
    dsh_d = nc.dram_tensor("dsh", [T, BL], f32, kind="ExternalInput").ap()
    out_d = nc.dram_tensor("out", [BL, Q, D], bf16, kind="ExternalOutput").ap()

    with tile.TileContext(nc) as tc:
        with (
            tc.tile_pool(name="const", bufs=1) as cpool,
            tc.tile_pool(name="apool", bufs=3) as apool,
            tc.tile_pool(name="wpool", bufs=2) as wpool,
            tc.tile_pool(name="outsb", bufs=3) as outpool,
            tc.tile_pool(name="psum", bufs=2, space="PSUM") as pspool,
        ):
            # --- per-core constants ---
            qb = cpool.tile([T, Q], f32)
            nc.scalar.dma_start(out=qb[:], in_=q_d.partition_broadcast(T))
            tshs = cpool.tile([T, BL], f32)
            nc.scalar.dma_start(out=tshs[:], in_=tsh_d[:, :])
            dshs = cpool.tile([T, BL], f32)
            nc.scalar.dma_start(out=dshs[:], in_=dsh_d[:, :])

            ecnt = 0
            for rep in range(reps):
                for b in range(BL):
                    af = apool.tile([T, D], f32r)
                    nc.gpsimd.dma_start(out=af[:], in_=a_d[b, :, :])

                    # clamp weights in two fused DVE ops (2x_2P)
                    w = wpool.tile([T, Q], f32r)
                    nc.vector.tensor_scalar(
                        out=w[:], in0=qb[:], scalar1=tshs[:, b : b + 1],
                        scalar2=0.0, op0=Alu.subtract, op1=Alu.max,
                    )
                    nc.vector.tensor_scalar(
                        out=w[:], in0=w[:], scalar1=dshs[:, b : b + 1],
                        scalar2=None, op0=Alu.min,
                    )

                    osb = outpool.tile([128, NQT * D], bf16)
                    for blk in range(NBLK):
                        ps = pspool.tile([128, PS_QT * D], f32)
                        for k2 in range(PS_QT):
                            qt = blk * PS_QT + k2
                            sl = slice(qt * 128, (qt + 1) * 128)
                            nc.tensor.matmul(
                                ps[:, k2 * D : (k2 + 1) * D],
                                lhsT=w[:, sl], rhs=af[:],
                                start=True, stop=True,
                            )
                        dst = osb[:, blk * PS_QT * D : (blk + 1) * PS_QT * D]
                        if ecnt % 16 in (0, 3, 6, 9, 12):
                            nc.vector.tensor_copy(out=dst, in_=ps[:])
                        else:
                            nc.scalar.copy(out=dst, in_=ps[:])
                        ecnt += 1
                    # one 2MiB DMA per b; each partition writes a 16KB run
                    dview = out_d[b].rearrange("(p c) d -> p (c d)", p=128)
                    nc.sync.dma_start(out=dview, in_=osb[:])
    nc.finalize()
    return nc


def _get_program(reps=1):
    global _PROGRAM
    if _PROGRAM is None:
        _PROGRAM = {}
    if reps not in _PROGRAM:
        _PROGRAM[reps] = _build_program(reps)
    return _PROGRAM[reps]


def kernel(query_t, knots, x0, x1, spline_discr, _trace=False, **_trace_kwargs):
    from concourse.bass_utils import run_bass_kernel_spmd

    qperm = permute_queries(query_t)
    A, tsh, dsh = host_prep(query_t, knots, x0, x1, spline_discr)

    nc = _get_program()
    in_maps = [
        make_core_inputs(qperm, A, tsh, dsh, c) for c in range(NCORES)
    ]
    res = run_bass_kernel_spmd(
        nc, in_maps, core_ids=list(range(NCORES)), trace=_trace, **_trace_kwargs
    )
    out = np.concatenate(
        [r["out"].astype(np.float32) for r in res.results], axis=0
    )
    if _trace:
        return out, res
    return out


# revision 9
# speedup vs baseline: 2.8021x; 1.2663x over previous
"""Trainium2 Bass kernel for nn_EndPointSpline.

Reference computation (per batch column b, feature d):
    xt = concat([x0, knots_b, x1])           # [T=128] knot values
    t  = spline_discr[:, b]                  # [T] sorted, t[0]=0, t[-1]=1
    vel[j] = (xt[j+1]-xt[j]) / (t[j+1]-t[j]+1e-10)
    left(q) = searchsorted(t[1:], q, 'left') clipped to [0, T-2]
    y(q) = xt[left] + vel[left] * (q - t[left])

Kernel strategy (data-parallel over B across 8 cores, 16 columns/core):
  Piecewise-linear interpolation in *integrated* form, normalized per
  segment so weights live in [0, 1]:
      y(q) = x0 + sum_j dxt[j] * clamp((q - t[j]) / dt'[j], 0, 1)
  with dxt[j] = xt[j+1]-xt[j] and dt'[j] = t[j+1]-t[j]+1e-10.  The
  weight matrix W'[i,q] is built in TWO fused DVE tensor_scalar ops
  (both 2x_2P fp32) over [128, 2048]:
      w = (q - tsh[i]) * rsh[i]     (row-0 sentinels tsh=-1, rsh=1)
      w = min(max(w, 0), 1)         (immediates; row 0 == 1 exactly)
  Saturated weights are EXACTLY 1.0, so fp16 W'/A cost only one
  boundary term of fp16 rounding per query: measured rel err 2.8e-3
  against the 2e-2 budget.  Row 0 of A holds x0; rows 1..127 hold dxt.
  One K=128 fp16 matmul per query tile gathers + lerps in one pass.

  The kernel is DMA-fabric-bound (436 GB/s SBUF-AXI per core), so all
  dtypes are chosen to minimize HBM bytes: A is fp16 (2 MiB/core), the
  device output is fp16 [BL, Q, D] (32 MiB/core, host upcasts to f32).
  PSUM (f32) evacuation is split DVE/ACT 5:11 (balancing the two DVE
  weight ops) and fully hides under the output DMA.

  Host-side marshalling: queries are permuted so each output partition
  writes a 16KB-contiguous DRAM run, giving ONE 2MiB output DMA per b
  (the kernel writes the output in ORIGINAL query order; only the
  compute order is permuted).
"""

import numpy as np

Q, B, T, D = 2048, 128, 128, 512
NCORES = 8
BL = B // NCORES          # 16 batch columns per core
K = T - 1                 # 127 segments
NQT = Q // 128            # 16 query tiles of 128
PS_QT = 4                 # query tiles per PSUM tile (4 banks)
NBLK = NQT // PS_QT       # evac blocks per b

_PROGRAM = None


def permute_queries(query_t):
    """qperm[k*128 + p] = query_t[p*NQT + k] so that output partition p
    holds queries p*16..p*16+15 (a 16KB fp16 DRAM run)."""
    a = np.asarray(query_t, dtype=np.float32).reshape(128, NQT)
    return np.ascontiguousarray(a.T.reshape(-1))


def host_prep(query_t, knots, x0, x1, spline_discr):
    """Everything that is shared across cores (f64 math, cast down)."""
    xt = np.concatenate(
        [
            np.asarray(x0, dtype=np.float32).transpose(1, 0, 2),
            np.asarray(knots, dtype=np.float32),
            np.asarray(x1, dtype=np.float32).transpose(1, 0, 2),
        ],
        axis=1,
    )                                                      # [B, T, D] f32
    t64 = np.asarray(spline_discr, dtype=np.float32).astype(np.float64)
    dtp64 = (t64[1:] - t64[:-1]) + 1e-10                   # [K, B]
    A = np.empty((B, T, D), np.float16)
    A[:, 0] = xt[:, 0]
    A[:, 1:] = xt[:, 1:, :] - xt[:, :-1, :]                # dxt

    # shifted t / reciprocal-dt with row-0 sentinels (-1 / +1 -> row-0
    # weight == clamp(q+1, 0, 1) == 1)
    tsh = np.empty((T, B), np.float32)
    tsh[0] = -1.0
    tsh[1:] = np.asarray(spline_discr, dtype=np.float32)[:K]
    rsh = np.empty((T, B), np.float32)
    rsh[0] = 1.0
    rsh[1:] = (1.0 / dtp64).astype(np.float32)
    return A, tsh, rsh


def make_core_inputs(qperm, A, tsh, rsh, core):
    s = slice(core * BL, (core + 1) * BL)
    return {
        "query_t": qperm,
        "amat": np.ascontiguousarray(A[s]),
        "tsh": np.ascontiguousarray(tsh[:, s]),
        "rsh": np.ascontiguousarray(rsh[:, s]),
    }


def _build_program(reps=1):
    import concourse.tile as tile
    from concourse import bacc, mybir

    f32 = mybir.dt.float32
    f16 = mybir.dt.float16
    Alu = mybir.AluOpType

    nc = bacc.Bacc("TRN2", target_bir_lowering=False, debug=False)

    q_d = nc.dram_tensor("query_t", [Q], f32, kind="ExternalInput").ap()
    a_d = nc.dram_tensor("amat", [BL, T, D], f16, kind="ExternalInput").ap()
    tsh_d = nc.dram_tensor("tsh", [T, BL], f32, kind="ExternalInput").ap()
    rsh_d = nc.dram_tensor("rsh", [T, BL], f32, kind="ExternalInput").ap()
    out_d = nc.dram_tensor("out", [BL, Q, D], f16, kind="ExternalOutput").ap()

    with tile.TileContext(nc) as tc:
        with (
            tc.tile_pool(name="const", bufs=1) as cpool,
            tc.tile_pool(name="apool", bufs=3) as apool,
            tc.tile_pool(name="wpool", bufs=2) as wpool,
            tc.tile_pool(name="outsb", bufs=4) as outpool,
            tc.tile_pool(name="psum", bufs=2, space="PSUM") as pspool,
        ):
            # --- per-core constants ---
            qb = cpool.tile([T, Q], f32)
            nc.scalar.dma_start(out=qb[:], in_=q_d.partition_broadcast(T))
            tshs = cpool.tile([T, BL], f32)
            nc.scalar.dma_start(out=tshs[:], in_=tsh_d[:, :])
            rshs = cpool.tile([T, BL], f32)
            nc.scalar.dma_start(out=rshs[:], in_=rsh_d[:, :])

            ecnt = 0
            for rep in range(reps):
                for b in range(BL):
                    af = apool.tile([T, D], f16)
                    nc.gpsimd.dma_start(out=af[:], in_=a_d[b, :, :])

                    # normalized clamp weights in two fused DVE ops (2x_2P)
                    wt = wpool.tile([T, Q], f32, tag="wtmp")
                    nc.vector.tensor_scalar(
                        out=wt[:], in0=qb[:], scalar1=tshs[:, b : b + 1],
                        scalar2=rshs[:, b : b + 1],
                        op0=Alu.subtract, op1=Alu.mult,
                    )
                    w = wpool.tile([T, Q], f16, tag="wf16")
                    nc.vector.tensor_scalar(
                        out=w[:], in0=wt[:], scalar1=0.0, scalar2=1.0,
                        op0=Alu.max, op1=Alu.min,
                    )

                    osb = outpool.tile([128, NQT * D], f16)
                    for blk in range(NBLK):
                        ps = pspool.tile([128, PS_QT * D], f32)
                        for k2 in range(PS_QT):
                            qt = blk * PS_QT + k2
                            sl = slice(qt * 128, (qt + 1) * 128)
                            nc.tensor.matmul(
                                ps[:, k2 * D : (k2 + 1) * D],
                                lhsT=w[:, sl], rhs=af[:],
                                start=True, stop=True,
                            )
                        dst = osb[:, blk * PS_QT * D : (blk + 1) * PS_QT * D]
                        if ecnt % 16 in (0, 3, 6, 9, 12):
                            nc.vector.tensor_copy(out=dst, in_=ps[:])
                        else:
                            nc.scalar.copy(out=dst, in_=ps[:])
                        ecnt += 1
                    # one 2MiB DMA per b; each partition writes a 16KB run
                    dview = out_d[b].rearrange("(p c) d -> p (c d)", p=128)
                    nc.sync.dma_start(out=dview, in_=osb[:])
    nc.finalize()
    return nc


def _get_program(reps=1):
    global _PROGRAM
    if _PROGRAM is None:
        _PROGRAM = {}
    if reps not in _PROGRAM:
        _PROGRAM[reps] = _build_program(reps)
    return _PROGRAM[reps]


def kernel(query_t, knots, x0, x1, spline_discr, _trace=False, **_trace_kwargs):
    from concourse.bass_utils import run_bass_kernel_spmd

    qperm = permute_queries(query_t)
    A, tsh, rsh = host_prep(query_t, knots, x0, x1, spline_discr)

    nc = _get_program()
    in_maps = [
        make_core_inputs(qperm, A, tsh, rsh, c) for c in range(NCORES)
    ]
    res = run_bass_kernel_spmd(
        nc, in_maps, core_ids=list(range(NCORES)), trace=_trace, **_trace_kwargs
    )
    out = np.concatenate(
        [r["out"].astype(np.float32) for r in res.results], axis=0
    )
    if _trace:
        return out, res
    return out
